# revision 1
# baseline (speedup 1.0000x reference)
"""AGNNProp on 8 Trainium2 NeuronCores.

out[i] = sum_{e: row_e = i} softmax_i(beta * cos(x_i, x_col_e)) * x[col_e]
with self-loops added (segment softmax grouped by destination row).

Strategy (graph/data parallel per sharding hint):
 - Host: normalize x (xhat = x/||x||); table rows = [xhat_bf16 | x_bf16]
   (512B, dma_gather-aligned) with two all-zero rows embedded at table
   positions 0 and 32767 so both int16 gather ranges can address a zero
   row for padding slots.  Group edges by destination, deal destinations
   round-robin (snake ordering by degree) to 8 cores, pad each
   128-destination tile to its max per-range degree.
 - Device, per destination tile: dma_gather of neighbor rows (desc-gen
   on GpSimd overlaps compute); cos = per-partition dot of xhat halves
   on DVE (2x bf16 mode); softmax without max-shift (beta*cos in [-1,1]);
   pad columns gather zero rows -> w=1, corrected by a host-sent pad
   count subtracted from the denominator (mask-free).  Weighted
   aggregation on the TensorEngine: stationary = strided diagonal-weight
   slices diagT[:, :, j] (built in one 2x-mode DVE multiply against a
   precomputed replicated identity), moving = raw-x halves of G; PSUM
   accumulates out[dest, feat] directly.  One batched xd load and one
   batched out store (partition-major DRAM layouts).
"""

import sys

sys.path.insert(0, "/opt/trn_rl_repo")

import numpy as np

N_NODES = 40000
N_EDGES = 640000
D = 128
NC = 8
P = 128
DPC = 5120  # padded destinations per core
TPC = DPC // P  # 40 tiles per core

# table layout: row 0 = zeros, rows 1..32766 = nodes 0..32765,
# row 32767 = zeros, rows 32768..40001 = nodes 32766..39999
TROWS = N_NODES + 2
ZROW_A = 0  # zero row reachable from gather A (table rows [0, 32768))
ZROW_B = 32767  # zero row reachable from gather B
B_BASE = TROWS - 32768  # 7234: gather B covers table rows [7234, 40002)
A_MAX_NODE = 32765  # nodes <= this are A-eligible (table row n+1 < 32768)
B_MIN_NODE = 7233  # nodes >= this are B-eligible (table row >= 7234)


def _trow(n):
    """Table row of node n (vectorized)."""
    n = np.asarray(n)
    return np.where(n <= A_MAX_NODE, n + 1, n + 2)


# ---------------------------------------------------------------- host side


def _preprocess(edge_index):
    """Per-destination neighbor lists, destination dealing, per-tile
    capacities with overlapping A/B gather ranges."""
    row = np.asarray(edge_index[0], dtype=np.int64)
    col = np.asarray(edge_index[1], dtype=np.int64)

    # sort edges by (dest, col): must-A cols first, flexible, must-B last
    perm = np.lexsort((col, row))
    row, col = row[perm], col[perm]

    deg = np.bincount(row, minlength=N_NODES)
    a_cnt = np.bincount(row[col < B_MIN_NODE], minlength=N_NODES)  # must-A
    b_cnt = np.bincount(row[col > A_MAX_NODE], minlength=N_NODES)  # must-B
    starts = np.zeros(N_NODES + 1, dtype=np.int64)
    np.cumsum(deg, out=starts[1:])

    # snake ordering: degree desc, a_cnt alternating asc/desc per group
    ud = np.sort(np.unique(deg))[::-1]
    parts = []
    for gi, dv in enumerate(ud):
        idx = np.where(deg == dv)[0]
        o2 = np.argsort(a_cnt[idx] if gi % 2 == 0 else -a_cnt[idx], kind="stable")
        parts.append(idx[o2])
    order = np.concatenate(parts)

    # deal: rank r -> core r%8, slot r//8;  pad slots get dest=-1
    dest = np.full((NC, DPC), -1, dtype=np.int64)
    for c in range(NC):
        got = order[c::NC]
        dest[c, : len(got)] = got

    # per-tile capacities + per-dest A counts (one SPMD graph: max over cores)
    KL = np.zeros(TPC, dtype=np.int64)
    KH = np.zeros(TPC, dtype=np.int64)
    nA = np.zeros(N_NODES, dtype=np.int64)  # cols assigned to gather A
    for t in range(TPC):
        d = dest[:, t * P : (t + 1) * P].ravel()
        d = d[d >= 0]
        if not len(d):
            continue
        kb1 = b_cnt[d].max()
        ka1 = np.maximum(a_cnt[d], deg[d] - kb1).max()
        ka2 = a_cnt[d].max()
        kb2 = np.maximum(b_cnt[d], deg[d] - ka2).max()
        if ka1 + kb1 <= ka2 + kb2:
            ka, kb = ka1, kb1
        else:
            ka, kb = ka2, kb2
        KL[t], KH[t] = ka, kb
        nA[d] = np.maximum(a_cnt[d], deg[d] - kb)
    return row, col, deg, nA, starts, dest, KL, KH


def _build_core_arrays(xhat, x, c, col, deg, nA, starts, dest, KL, KH):
    """Per-core input arrays (vectorized): packed int16 gather indices,
    pad counts, destination features [xhat | x] partition-major."""
    import ml_dtypes

    KT = KL + KH
    WLO = int((KL * 8).sum())
    WHI = int((KH * 8).sum())
    olo8 = np.zeros(TPC, dtype=np.int64)
    ohi8 = np.zeros(TPC, dtype=np.int64)
    np.cumsum(KL[:-1] * 8, out=olo8[1:])
    np.cumsum(KH[:-1] * 8, out=ohi8[1:])

    # pad slots gather the zero rows
    idxlo = np.full((16, max(WLO, 1)), ZROW_A, dtype=np.int16)
    idxhi = np.full((16, max(WHI, 1)), ZROW_B - B_BASE, dtype=np.int16)
    npad = np.zeros((P, TPC), dtype=np.float32)
    xd = np.zeros((P, TPC, 256), dtype=ml_dtypes.bfloat16)

    slots = np.arange(DPC)
    d_all = dest[c]
    valid = d_all >= 0
    tt, pp = slots // P, slots % P

    # npad[p, t] = kt - deg (live) or kt (pad dest); +1 if diagT pad col
    npad[pp, tt] = KT[tt]
    npad[pp[valid], tt[valid]] -= deg[d_all[valid]]
    npad[:, :] += ((KT + 1) % 2)[None, :]  # ks2 rounding pad column

    xd[pp[valid], tt[valid], 0:D] = xhat[d_all[valid]].astype(ml_dtypes.bfloat16)
    xd[pp[valid], tt[valid], D : 2 * D] = x[d_all[valid]].astype(
        ml_dtypes.bfloat16
    )

    dv = d_all[valid]
    tv, pv = tt[valid], pp[valid]
    na = nA[dv]  # A-count per dest
    nb = deg[dv] - na

    # --- gather A entries: per dest, cols[starts[d] : starts[d]+na]
    repA = np.repeat(np.arange(len(dv)), na)
    jA = np.arange(repA.size) - np.repeat(np.cumsum(na) - na, na)
    eA = np.repeat(starts[dv], na) + jA
    iA = jA * P + pv[repA]
    cA = olo8[tv[repA]] + iA // 16
    idxlo[iA % 16, cA] = _trow(col[eA]).astype(np.int16)

    # --- gather B entries: per dest, cols[starts[d]+na : starts[d]+deg]
    repB = np.repeat(np.arange(len(dv)), nb)
    jB = np.arange(repB.size) - np.repeat(np.cumsum(nb) - nb, nb)
    eB = np.repeat(starts[dv] + na, nb) + jB
    iB = jB * P + pv[repB]
    cB = ohi8[tv[repB]] + iB // 16
    idxhi[iB % 16, cB] = (_trow(col[eB]) - B_BASE).astype(np.int16)

    return np.tile(idxlo, (8, 1)), np.tile(idxhi, (8, 1)), npad, xd


# ------------------------------------------------------------- device side


def _build_graph(KL, KH, WLO, WHI, KSMAX):
    import concourse.bass as bass
    import concourse.mybir as mybir
    import concourse.tile as tile
    from concourse import bacc
    from concourse.masks import make_identity

    f32 = mybir.dt.float32
    bf16 = mybir.dt.bfloat16
    i16 = mybir.dt.int16
    AF = mybir.ActivationFunctionType
    OP = mybir.AluOpType

    nc = bacc.Bacc(num_swdge_queues=2)
    tab_ext = nc.declare_dram_parameter("tab", [TROWS, 256], bf16, isOutput=False)
    xd_ext = nc.declare_dram_parameter("xd", [P, TPC * 256], bf16, isOutput=False)
    ilo_ext = nc.declare_dram_parameter("idxlo", [P, max(WLO, 8)], i16, isOutput=False)
    ihi_ext = nc.declare_dram_parameter("idxhi", [P, max(WHI, 8)], i16, isOutput=False)
    npad_ext = nc.declare_dram_parameter("npad", [P, TPC], f32, isOutput=False)
    beta_ext = nc.declare_dram_parameter("beta2", [P, 2], f32, isOutput=False)
    out_ext = nc.declare_dram_parameter("out", [P, TPC * P], f32, isOutput=True)

    with tile.TileContext(nc) as tc:
        with (
            tc.tile_pool(name="persist", bufs=1) as pp,
            tc.tile_pool(name="gather", bufs=5) as pg,
            tc.tile_pool(name="scr", bufs=2) as psc,
            tc.tile_pool(name="diag", bufs=2) as pd,
            tc.tile_pool(name="small", bufs=3) as psm,
            tc.tile_pool(name="psum", bufs=4, space="PSUM") as pps,
        ):
            betat = pp.tile([P, 2], f32)
            nc.sync.dma_start(out=betat[:], in_=beta_ext[:])
            npadt = pp.tile([P, TPC], f32)
            nc.sync.dma_start(out=npadt[:], in_=npad_ext[:])
            ilo_all = pp.tile([P, max(WLO, 8)], i16)
            nc.sync.dma_start(out=ilo_all[:], in_=ilo_ext[:])
            ihi_all = pp.tile([P, max(WHI, 8)], i16)
            nc.sync.dma_start(out=ihi_all[:], in_=ihi_ext[:])
            xds = pp.tile([P, TPC, 256], bf16)
            nc.sync.dma_start(
                out=xds[:], in_=xd_ext[:].rearrange("p (t e) -> p t e", t=TPC)
            )

            ident = pp.tile([P, P], bf16)
            make_identity(nc, ident[:])
            identrep = pp.tile([P, P, KSMAX], bf16)
            nc.vector.tensor_copy(
                identrep[:], ident[:, :, None].broadcast_to([P, P, KSMAX])
            )

            outacc = pp.tile([P, TPC, P], f32)

            olo = ohi = 0
            for t in range(TPC):
                kl, kh = int(KL[t]), int(KH[t])
                kt = kl + kh
                ks = kt + 1  # + self column
                ks2 = ks + (ks % 2)  # even width for 4B-aligned diagT rows
                if kt:
                    G = pg.tile([P, kt, 256], bf16, tag="G")
                    if kl:
                        nc.gpsimd.dma_gather(
                            G[:, 0:kl, :], tab_ext[0:32768, :],
                            ilo_all[:, olo : olo + kl * 8],
                            P * kl, P * kl, 256, single_packet=False,
                            queue_num=0,
                        )
                    if kh:
                        nc.gpsimd.dma_gather(
                            G[:, kl:kt, :], tab_ext[B_BASE:, :],
                            ihi_all[:, ohi : ohi + kh * 8],
                            P * kh, P * kh, 256, single_packet=False,
                            queue_num=1,
                        )

                cosm = psm.tile([P, ks2], f32, tag="cosm")
                nc.vector.memset(cosm[:, kt:ks2], 0.0)  # pad col (if any)
                nc.vector.memset(cosm[:, kt : kt + 1], 1.0)  # self cos
                if kt:
                    prod = psc.tile([P, kt, D], bf16, tag="prod")
                    nc.vector.tensor_tensor(
                        out=prod[:],
                        in0=G[:, :, 0:D],
                        in1=xds[:, t, 0:D][:, None, :].broadcast_to([P, kt, D]),
                        op=OP.mult,
                    )
                    nc.vector.tensor_reduce(
                        out=cosm[:, 0:kt], in_=prod[:],
                        axis=mybir.AxisListType.X, op=OP.add,
                    )
                # softmax over ks2 columns (no max shift: beta*cos in [-1,1])
                w = psm.tile([P, ks2], f32, tag="w")
                nc.scalar.activation(w[:], cosm[:], AF.Exp, scale=betat[:, 0:1])
                dn = psm.tile([P, 1], f32, tag="dn")
                nc.vector.tensor_reduce(
                    out=dn[:], in_=w[:], axis=mybir.AxisListType.X, op=OP.add
                )
                dn2 = psm.tile([P, 1], f32, tag="dn2")
                nc.vector.tensor_scalar(
                    out=dn2[:], in0=dn[:], scalar1=npadt[:, t : t + 1],
                    scalar2=None, op0=OP.subtract,
                )
                ivn = psm.tile([P, 1], f32, tag="ivn")
                nc.vector.reciprocal(ivn[:], dn2[:])
                wnb = psm.tile([P, ks2], bf16, tag="wnb")
                nc.vector.tensor_scalar(
                    out=wnb[:], in0=w[:], scalar1=ivn[:], scalar2=None,
                    op0=OP.mult,
                )
                # diagT[p, d, j] = ident[p, d] * wn[p, j]  (2x bf16 mode:
                # broadcast sits on the middle dim, innermost is contiguous)
                diagT = pd.tile([P, P, ks2], bf16, tag="diagT")
                nc.vector.tensor_tensor(
                    out=diagT[:],
                    in0=identrep[:, :, 0:ks2],
                    in1=wnb[:, None, :].broadcast_to([P, P, ks2]),
                    op=OP.mult,
                )
                # aggregation: ps[d, f] += sum_e diagT[e, d, j] * Gx[e, j, f]
                ps = pps.tile([P, P], f32)
                for j in range(kt):
                    nc.tensor.matmul(
                        out=ps[:], lhsT=diagT[:, :, j], rhs=G[:, j, D : 2 * D],
                        start=(j == 0), stop=False,
                    )
                nc.tensor.matmul(
                    out=ps[:], lhsT=diagT[:, :, kt], rhs=xds[:, t, D : 2 * D],
                    start=(kt == 0), stop=True,
                )
                nc.scalar.copy(outacc[:, t, :], ps[:])

                olo += kl * 8
                ohi += kh * 8

            nc.sync.dma_start(
                out=out_ext[:].rearrange("p (t f) -> p t f", t=TPC),
                in_=outacc[:],
            )
    nc.finalize()
    return nc


# ----------------------------------------------------------------- entry


def kernel(x, beta, edge_index):
    import ml_dtypes

    from concourse.bass_utils import run_bass_kernel_spmd

    x = np.asarray(x, dtype=np.float32)
    beta = np.asarray(beta, dtype=np.float32)

    norm = np.sqrt((x * x).sum(axis=1))
    xhat = x / norm[:, None]

    row, col, deg, nA, starts, dest, KL, KH = _preprocess(edge_index)
    KT = KL + KH
    WLO = int((KL * 8).sum())
    WHI = int((KH * 8).sum())
    KSMAX = int(KT.max()) + 2
    KSMAX += KSMAX % 2  # even: keeps identrep rows 4B-aligned

    beta2 = np.zeros((P, 2), dtype=np.float32)
    beta2[:, 0] = beta[0]

    tab = np.zeros((TROWS, 256), dtype=ml_dtypes.bfloat16)
    xhat_bf = xhat.astype(ml_dtypes.bfloat16)
    x_bf = x.astype(ml_dtypes.bfloat16)
    tab[1 : A_MAX_NODE + 2, 0:D] = xhat_bf[: A_MAX_NODE + 1]
    tab[1 : A_MAX_NODE + 2, D : 2 * D] = x_bf[: A_MAX_NODE + 1]
    tab[ZROW_B + 1 :, 0:D] = xhat_bf[A_MAX_NODE + 1 :]
    tab[ZROW_B + 1 :, D : 2 * D] = x_bf[A_MAX_NODE + 1 :]

    in_maps = []
    for c in range(NC):
        idxlo, idxhi, npad, xd = _build_core_arrays(
            xhat, x, c, col, deg, nA, starts, dest, KL, KH
        )
        if WLO == 0:
            idxlo = np.zeros((P, 8), dtype=np.int16)
        if WHI == 0:
            idxhi = np.zeros((P, 8), dtype=np.int16)
        in_maps.append(
            {
                "tab": tab,
                "xd": xd.reshape(P, TPC * 256),
                "idxlo": idxlo,
                "idxhi": idxhi,
                "npad": npad,
                "beta2": beta2,
            }
        )

    nc = _build_graph(KL, KH, WLO, WHI, KSMAX)
    import os

    trace = bool(int(os.environ.get("KERNEL_TRACE", "0")))
    res = run_bass_kernel_spmd(
        nc, in_maps, core_ids=list(range(NC)), trace=trace
    )
    global _last_results
    _last_results = res

    out = np.zeros((N_NODES, D), dtype=np.float32)
    for c in range(NC):
        o = res.results[c]["out"].reshape(P, TPC, P)  # [dest_p, t, feat]
        for t in range(TPC):
            d = dest[c, t * P : (t + 1) * P]
            live = d >= 0
            out[d[live]] = o[live, t, :]
    return out


if __name__ == "__main__":
    sys.path.insert(0, "/root/problem")
    import reference

    inputs = {k: np.asarray(v) for k, v in reference.setup_inputs().items()}
    expected = np.asarray(reference.reference(**inputs))
    actual = kernel(**inputs)
    rel = np.linalg.norm(actual - expected) / np.linalg.norm(expected)
    print("rel:", rel)



# revision 2
# speedup vs baseline: 2.1628x; 2.1628x over previous
"""AGNNProp on 8 Trainium2 NeuronCores.

out[i] = sum_{e: row_e = i} softmax_i(beta * cos(x_i, x_col_e)) * x[col_e]
with self-loops added (segment softmax grouped by destination row).

Strategy (graph/data parallel per sharding hint), v1 "host pre-gather":
 - Host: sort edges by destination, deal destinations round-robin in
   degree-sorted order to 8 cores (so each 128-dest tile has uniform
   degree -> little padding).  For each core, build dest-major gathered
   arrays: gx[p, (t,j), f] = x[col of j-th edge of dest p in tile t]
   (bf16, self-loop slot at j=deg, zero rows for pads), per-slot
   inverse-norm products ivn2 = 1/(|x_d||x_c|), dest features xd, and
   pad counts.  No device-side dma_gather at all (the baseline's
   bottleneck was GpSimd descriptor generation at ~5.4ns/edge).
 - Device, per tile: DVE dot product (mult at 2x bf16 + binary tree
   adds at 2x + short 1x tensor_reduce tail; avoids the 1x-only full
   tensor_reduce), cos = dot * ivn2; ScalarE exp with fused accum_out
   denominator; DVE builds diag-weight slices (ident * w broadcast);
   TensorE aggregates via diag matmuls into PSUM; ScalarE drains PSUM
   with the 1/Z softmax normalization folded into the copy scale.
"""

import sys

sys.path.insert(0, "/opt/trn_rl_repo")

import numpy as np

N_NODES = 40000
N_EDGES = 640000
D = 128
NC = 8
P = 128
DPC = 5120  # padded destinations per core
TPC = DPC // P  # 40 tiles per core


# ---------------------------------------------------------------- host side


def _preprocess(edge_index):
    """Sort edges by dest, deal destinations to cores in degree-sorted
    order, compute shared per-tile capacities K[t] (incl. self slot)."""
    row = np.asarray(edge_index[0], dtype=np.int64)
    col = np.asarray(edge_index[1], dtype=np.int64)

    perm = np.argsort(row, kind="stable")
    row, col = row[perm], col[perm]

    deg = np.bincount(row, minlength=N_NODES)
    starts = np.zeros(N_NODES + 1, dtype=np.int64)
    np.cumsum(deg, out=starts[1:])

    order = np.argsort(-deg, kind="stable")

    # deal: rank r -> core r%8, slot r//8; pad slots get dest=-1
    dest = np.full((NC, DPC), -1, dtype=np.int64)
    for c in range(NC):
        got = order[c::NC]
        dest[c, : len(got)] = got

    # shared per-tile capacity: max (deg+1) over the pooled dests of all
    # cores (one SPMD graph for all 8 cores)
    K = np.zeros(TPC, dtype=np.int64)
    for t in range(TPC):
        d = dest[:, t * P : (t + 1) * P].ravel()
        d = d[d >= 0]
        K[t] = (deg[d].max() + 1) if len(d) else 1
    return col, deg, starts, dest, K


def _build_core_arrays(c, col, deg, starts, dest, K, koff, SK, x_bf, invn):
    """Per-core input arrays: src_map -> gathered gx, ivn2, xd, npad."""
    import ml_dtypes

    d_all = dest[c]  # [DPC]
    valid = d_all >= 0
    slots = np.arange(DPC)
    tt, pp = slots // P, slots % P

    # src[p, s] = source node of slot (tile t, j) for dest p; -1 = pad
    src = np.full((P, SK), -1, dtype=np.int64)
    dcol = np.full((P, SK), -1, dtype=np.int64)  # dest per slot (live)

    dv = d_all[valid]
    tv, pv = tt[valid], pp[valid]

    # edges: for each live dest, j in [0, deg)
    nd = deg[dv]
    repi = np.repeat(np.arange(len(dv)), nd)
    j = np.arange(repi.size) - np.repeat(np.cumsum(nd) - nd, nd)
    e = np.repeat(starts[dv], nd) + j
    src[pv[repi], koff[tv[repi]] + j] = col[e]
    dcol[pv[repi], koff[tv[repi]] + j] = dv[repi]
    # self slot at j = deg
    src[pv, koff[tv] + nd] = dv
    dcol[pv, koff[tv] + nd] = dv

    # gathered features (row 0 of the +1-shifted table = zeros)
    xtab = np.zeros((N_NODES + 1, D), dtype=ml_dtypes.bfloat16)
    xtab[1:] = x_bf
    gx = xtab[src + 1]  # [P, SK, D] bf16

    itab = np.zeros(N_NODES + 1, dtype=np.float32)
    itab[1:] = invn
    ivn2 = itab[src + 1] * itab[dcol + 1]  # [P, SK] f32 (0 for pads)

    xd = np.zeros((P, TPC, D), dtype=ml_dtypes.bfloat16)
    xd[pv, tv] = x_bf[dv]

    npad = np.zeros((P, TPC), dtype=np.float32)
    npad[pp, tt] = K[tt]
    npad[pv, tv] -= deg[dv] + 1
    return gx, ivn2, xd, npad


# ------------------------------------------------------------- device side


def _build_graph(K, koff, SK, KMAX):
    import concourse.bass as bass
    import concourse.mybir as mybir
    import concourse.tile as tile
    from concourse import bacc
    from concourse.masks import make_identity

    f32 = mybir.dt.float32
    bf16 = mybir.dt.bfloat16
    AF = mybir.ActivationFunctionType
    OP = mybir.AluOpType

    nc = bacc.Bacc()
    gx_ext = nc.declare_dram_parameter("gx", [P, SK * D], bf16, isOutput=False)
    xd_ext = nc.declare_dram_parameter("xd", [P, TPC * D], bf16, isOutput=False)
    ivn2_ext = nc.declare_dram_parameter("ivn2", [P, SK], f32, isOutput=False)
    npad_ext = nc.declare_dram_parameter("npad", [P, TPC], f32, isOutput=False)
    beta_ext = nc.declare_dram_parameter("beta2", [P, 2], f32, isOutput=False)
    out_ext = nc.declare_dram_parameter("out", [P, TPC * D], f32, isOutput=True)

    gx3 = gx_ext[:].rearrange("p (s f) -> p s f", f=D)

    with tile.TileContext(nc) as tc:
        with (
            tc.tile_pool(name="persist", bufs=1) as pp,
            tc.tile_pool(name="gx", bufs=4) as pg,
            tc.tile_pool(name="prod", bufs=2) as ppr,
            tc.tile_pool(name="tree", bufs=2) as ptr,
            tc.tile_pool(name="diag", bufs=2) as pd,
            tc.tile_pool(name="small", bufs=3) as psm,
            tc.tile_pool(name="psum", bufs=4, space="PSUM") as pps,
        ):
            betat = pp.tile([P, 2], f32)
            nc.sync.dma_start(out=betat[:], in_=beta_ext[:])
            npadt = pp.tile([P, TPC], f32)
            nc.sync.dma_start(out=npadt[:], in_=npad_ext[:])
            ivn2s = pp.tile([P, SK], f32)
            nc.sync.dma_start(out=ivn2s[:], in_=ivn2_ext[:])
            xds = pp.tile([P, TPC, D], bf16)
            nc.sync.dma_start(
                out=xds[:], in_=xd_ext[:].rearrange("p (t f) -> p t f", t=TPC)
            )

            ident = pp.tile([P, P], bf16)
            make_identity(nc, ident[:])
            identrep = pp.tile([P, P, KMAX], bf16)
            nc.vector.tensor_copy(
                identrep[:], ident[:, :, None].broadcast_to([P, P, KMAX])
            )

            ws = pp.tile([P, SK], bf16)  # exp weights, all tiles
            dn = pp.tile([P, TPC], f32)  # softmax denominators (w/ pads)
            outacc = pp.tile([P, TPC, D], f32)

            for t in range(TPC):
                k = int(K[t])
                so = int(koff[t])
                gxt = pg.tile([P, KMAX, D], bf16, tag="gx")
                nc.sync.dma_start(out=gxt[:, 0:k, :], in_=gx3[:, so : so + k, :])

                # dot[p, j] = sum_f gx[p,j,f] * xd[p,f]  (2x bf16 DVE)
                prod = ppr.tile([P, KMAX, D], bf16, tag="prod")
                nc.vector.tensor_tensor(
                    out=prod[:, 0:k, :],
                    in0=gxt[:, 0:k, :],
                    in1=xds[:, t, None, :].broadcast_to([P, k, D]),
                    op=OP.mult,
                )
                t64 = ptr.tile([P, KMAX, 64], bf16, tag="t64")
                nc.vector.tensor_tensor(
                    out=t64[:, 0:k, :], in0=prod[:, 0:k, 0:64],
                    in1=prod[:, 0:k, 64:128], op=OP.add,
                )
                t32 = ptr.tile([P, KMAX, 32], bf16, tag="t32")
                nc.vector.tensor_tensor(
                    out=t32[:, 0:k, :], in0=t64[:, 0:k, 0:32],
                    in1=t64[:, 0:k, 32:64], op=OP.add,
                )
                t16 = ptr.tile([P, KMAX, 16], bf16, tag="t16")
                nc.vector.tensor_tensor(
                    out=t16[:, 0:k, :], in0=t32[:, 0:k, 0:16],
                    in1=t32[:, 0:k, 16:32], op=OP.add,
                )
                dot = psm.tile([P, KMAX], f32, tag="dot")
                nc.vector.tensor_reduce(
                    out=dot[:, 0:k], in_=t16[:, 0:k, :],
                    axis=mybir.AxisListType.X, op=OP.add,
                )
                # cos = dot * ivn2 (pads: ivn2 = 0 -> cos 0 -> w 1)
                cosm = psm.tile([P, KMAX], f32, tag="cosm")
                nc.vector.tensor_tensor(
                    out=cosm[:, 0:k], in0=dot[:, 0:k],
                    in1=ivn2s[:, so : so + k], op=OP.mult,
                )
                # w = exp(beta*cos); accum_out = sum_j w (incl pads)
                nc.scalar.activation(
                    ws[:, so : so + k], cosm[:, 0:k], AF.Exp,
                    scale=betat[:, 0:1], accum_out=dn[:, t : t + 1],
                )

                # diagT[p, d, j] = ident[p, d] * w[p, j]
                diagT = pd.tile([P, P, KMAX], bf16, tag="diagT")
                nc.vector.tensor_tensor(
                    out=diagT[:, :, 0:k],
                    in0=identrep[:, :, 0:k],
                    in1=ws[:, None, so : so + k].broadcast_to([P, P, k]),
                    op=OP.mult,
                )
                # ps[d, f] = sum_j w[d, j] * gx[d, j, f]
                ps = pps.tile([P, D], f32)
                for j in range(k):
                    nc.tensor.matmul(
                        out=ps[:], lhsT=diagT[:, :, j], rhs=gxt[:, j, :],
                        start=(j == 0), stop=(j == k - 1),
                    )
                # Z = dn - npad; out = ps / Z (scale on the PSUM drain)
                z = psm.tile([P, 1], f32, tag="z")
                nc.vector.tensor_scalar(
                    out=z[:], in0=dn[:, t : t + 1],
                    scalar1=npadt[:, t : t + 1], scalar2=None,
                    op0=OP.subtract,
                )
                ivz = psm.tile([P, 1], f32, tag="ivz")
                nc.vector.reciprocal(ivz[:], z[:])
                nc.scalar.activation(
                    outacc[:, t, :], ps[:], AF.Copy, scale=ivz[:, 0:1]
                )

            nc.sync.dma_start(
                out=out_ext[:].rearrange("p (t f) -> p t f", t=TPC),
                in_=outacc[:],
            )
    nc.finalize()
    return nc


# ----------------------------------------------------------------- entry


def kernel(x, beta, edge_index):
    import ml_dtypes

    from concourse.bass_utils import run_bass_kernel_spmd

    x = np.asarray(x, dtype=np.float32)
    beta = np.asarray(beta, dtype=np.float32)

    norm = np.sqrt((x * x).sum(axis=1))
    invn = (1.0 / norm).astype(np.float32)
    x_bf = x.astype(ml_dtypes.bfloat16)

    col, deg, starts, dest, K = _preprocess(edge_index)
    koff = np.zeros(TPC, dtype=np.int64)
    np.cumsum(K[:-1], out=koff[1:])
    SK = int(K.sum())
    KMAX = int(K.max())

    beta2 = np.zeros((P, 2), dtype=np.float32)
    beta2[:, 0] = beta[0]

    in_maps = []
    for c in range(NC):
        gx, ivn2, xd, npad = _build_core_arrays(
            c, col, deg, starts, dest, K, koff, SK, x_bf, invn
        )
        in_maps.append(
            {
                "gx": gx.reshape(P, SK * D),
                "xd": xd.reshape(P, TPC * D),
                "ivn2": ivn2,
                "npad": npad,
                "beta2": beta2,
            }
        )

    nc = _build_graph(K, koff, SK, KMAX)
    import os

    trace = bool(int(os.environ.get("KERNEL_TRACE", "0")))
    res = run_bass_kernel_spmd(
        nc, in_maps, core_ids=list(range(NC)), trace=trace
    )
    global _last_results
    _last_results = res

    out = np.zeros((N_NODES, D), dtype=np.float32)
    for c in range(NC):
        o = res.results[c]["out"].reshape(P, TPC, D)  # [dest_p, t, feat]
        d = dest[c]
        live = d >= 0
        tt, pp = np.arange(DPC) // P, np.arange(DPC) % P
        out[d[live]] = o[pp[live], tt[live], :]
    return out


if __name__ == "__main__":
    sys.path.insert(0, "/root/problem")
    import reference

    inputs = {k: np.asarray(v) for k, v in reference.setup_inputs().items()}
    expected = np.asarray(reference.reference(**inputs))
    actual = kernel(**inputs)
    rel = np.linalg.norm(actual - expected) / np.linalg.norm(expected)
    print("rel:", rel)
